# revision 20
# baseline (speedup 1.0000x reference)
"""GQA attention block (B=1, S=2048, HID=2048, NH=32, NKV=8, DH=64) on 8 trn2
NeuronCores — v4.

Sharding: tensor-parallel over heads (core c: query heads [4c,4c+4), KV head
c), then a sequence re-shard for the output projection.

v4 over v3:
- startup: the first projection matmul's inputs (wq/wkv/hs for kt 0-1) load
  first as small DMAs, so compute starts ~7us earlier.
- RoPE rotate-half is a partition-permute DMA of u = q*ssin (sign-and-partner
  folded into the ssin table) instead of a PE matmul; V transposes use the
  DMA transpose engine. cos/ssin ship as bf16 [128,S]. The PE only does
  projections, scores (+causal diag), AV, and the output projection.
- output-projection bias rides the PSUM->SBUF eviction as a DVE add with a
  host-broadcast bias table (no ones/bias matmuls).
- attention runs i-chunks in order [2,3,0,1]: the {2,3} AllToAll fires at
  ~55% of attention and lands mid-phase; the {0,1} AllToAll flies while the
  {2,3} output projection runs, so no collective is exposed.
- a2a stores coalesce both head-pairs per (ic, d-half) with 256B dram runs;
  the a2a output reloads as one DMA per pair with 512B runs.
- wo streams in 2MB slices during the projection phase to smooth DMA power.

All matmul inputs are bf16 (fp32 PSUM accumulation). attention_mask must be
all-ones (the problem spec fill); only the causal mask is applied.
"""

import sys

sys.path.insert(0, "/opt/trn_rl_repo")

import numpy as np
import ml_dtypes

import concourse.bacc as bacc
import concourse.mybir as mybir
import concourse.tile as tile
from concourse.bass_utils import run_bass_kernel_spmd

F32 = mybir.dt.float32
BF16 = mybir.dt.bfloat16
Exp = mybir.ActivationFunctionType.Exp

N_CORES = 8
S = 2048
HID = 2048
NH, NKV, DH = 32, 8, 64
NH_C = NH // N_CORES          # 4 query heads per core
P = 128
SC = 512                      # i-chunk (attention free dim)
N_SC = S // SC                # 4
KT = HID // P                 # 16 contraction tiles
ST = S // P                   # 16 key tiles of 128
SCALE = 1.0 / np.sqrt(DH)
SSH = S // N_CORES            # 256 output rows per core
HH = DH // 2                  # 32, rotate-half block

last_results = None


def _build():
    nc = bacc.Bacc("TRN2", target_bir_lowering=False, debug=False,
                   num_devices=N_CORES)

    # ---- kernel I/O ----
    hsp_d = nc.dram_tensor("hsp", [N_SC, P, KT * SC], BF16, kind="ExternalInput")
    wqp_d = nc.dram_tensor("wqp", [P, KT * NH_C * DH], BF16, kind="ExternalInput")
    wkvp_d = nc.dram_tensor("wkvp", [P, KT * 2 * DH], BF16, kind="ExternalInput")
    bv_d = nc.dram_tensor("bv", [P, 1], F32, kind="ExternalInput")
    cos_d = nc.dram_tensor("cos2", [P, S], BF16, kind="ExternalInput")
    ssin_d = nc.dram_tensor("ssin2", [P, S], BF16, kind="ExternalInput")
    tneg_d = nc.dram_tensor("tneg", [P, P], BF16, kind="ExternalInput")
    ident2_d = nc.dram_tensor("ident2", [P, 2 * P], BF16, kind="ExternalInput")
    vpad_d = nc.dram_tensor("vpad", [P, ST, DH], BF16, kind="ExternalInput")
    wop_d = nc.dram_tensor("wop", [P, KT * HID], BF16, kind="ExternalInput")
    bob_d = nc.dram_tensor("bob", [P, HID], BF16, kind="ExternalInput")
    out_d = nc.dram_tensor("out", [SSH, HID], F32, kind="ExternalOutput")

    # sequence re-shard: two AllToAlls over [e][p][t][q][s] blocks.
    # pr=1 carries i-chunks {2,3} (fires mid-attention), pr=0 carries {0,1}
    # (fires last, overlapped by the pr=1 output projection).
    a2a_in = [nc.dram_tensor(f"a2a_in{i}", [N_CORES, P, 2, 2, DH], BF16)
              for i in range(2)]
    a2a_out = [nc.dram_tensor(f"a2a_out{i}", [N_CORES, P, 2, 2, DH], BF16)
               for i in range(2)]
    warm_in = nc.dram_tensor("warm_in", [N_CORES, 1, 2], BF16)
    warm_out = nc.dram_tensor("warm_out", [N_CORES, 1, 2], BF16)
    grp = [list(range(N_CORES))]

    with tile.TileContext(nc) as tc:
        # fire the collective rendezvous barrier immediately so it overlaps
        # the projection phase instead of the first real AllToAll
        nc.gpsimd.collective_compute(
            "AllToAll", mybir.AluOpType.bypass, replica_groups=grp,
            ins=[warm_in[:]], outs=[warm_out[:]])

        with tc.tile_pool(name="persist", bufs=1) as persist:
            # dummy broadcast so the gpsimd extended library loads during the
            # initial DMA phase, not at the first normalization
            warmbc = persist.tile([P, 8], BF16)
            nc.gpsimd.partition_broadcast(warmbc[:], warmbc[0:1, :])
            wq_sb = persist.tile([P, KT, NH_C * DH], BF16)
            wkv_sb = persist.tile([P, KT, 2 * DH], BF16)
            cos_sb = persist.tile([P, S], BF16)
            ssin_sb = persist.tile([P, S], BF16)
            tneg_sb = persist.tile([P, P], BF16)
            ident2_sb = persist.tile([P, 2, P], BF16)
            bv_sb = persist.tile([P, 1], F32)
            bob_sb = persist.tile([P, HID], BF16)

            # ---- persistent activations ----
            qT_sb = persist.tile([P, 2, S], BF16)      # 4 heads, 2 per tile
            kT_sb = persist.tile([P, S], BF16)         # rows 0:64 kT, 64:128 dup
            v_aug = persist.tile([P, ST, P], BF16)     # [s, s-tile, ones+pad+v]
            wo_sb = persist.tile([P, KT, HID], BF16)

            wqp_r = wqp_d.rearrange("p (kt m) -> p kt m", kt=KT)
            wkvp_r = wkvp_d.rearrange("p (kt m) -> p kt m", kt=KT)
            wop_r = wop_d.rearrange("p (jt n) -> p jt n", jt=KT)

            # ================= QKV projection + RoPE =================
            with tc.tile_pool(name="hs", bufs=2) as hs_pool, \
                 tc.tile_pool(name="proj_ps", bufs=6, space="PSUM") as proj_ps, \
                 tc.tile_pool(name="rope", bufs=4) as rope_pool:

                def emit_rope_q(m, ps_q, ss, sc):
                    # qT = q*cos + perm(q*ssin); perm swaps 32-row halves
                    # within each 64-block (sign folded into ssin)
                    qcos = rope_pool.tile([P, SC], F32, tag="qcos",
                                          name=f"qcos_{sc}_{m}")
                    nc.vector.tensor_mul(qcos[:], ps_q[:], cos_sb[:, ss])
                    u = rope_pool.tile([P, SC], BF16, tag="u",
                                       name=f"u_{sc}_{m}")
                    nc.vector.tensor_mul(u[:], ps_q[:], ssin_sb[:, ss])
                    up = rope_pool.tile([P, SC], BF16, tag="up",
                                        name=f"up_{sc}_{m}")
                    for blk in (0, DH):
                        nc.scalar.dma_start(up[blk:blk + HH, :],
                                            u[blk + HH:blk + DH, :])
                        nc.scalar.dma_start(up[blk + HH:blk + DH, :],
                                            u[blk:blk + HH, :])
                    nc.vector.tensor_add(qT_sb[:, m, ss], qcos[:], up[:])

                def emit_rope_k(ps_kv, ss, sc):
                    kcos = rope_pool.tile([DH, SC], F32, tag="kcos",
                                          name=f"kcos_{sc}")
                    nc.vector.tensor_mul(kcos[:], ps_kv[0:DH, :],
                                         cos_sb[0:DH, ss])
                    uk = rope_pool.tile([DH, SC], BF16, tag="uk",
                                        name=f"uk_{sc}")
                    nc.vector.tensor_mul(uk[:], ps_kv[0:DH, :],
                                         ssin_sb[0:DH, ss])
                    ukp = rope_pool.tile([DH, SC], BF16, tag="ukp",
                                         name=f"ukp_{sc}")
                    nc.scalar.dma_start(ukp[0:HH, :], uk[HH:DH, :])
                    nc.scalar.dma_start(ukp[HH:DH, :], uk[0:HH, :])
                    nc.vector.tensor_add(kT_sb[0:DH, ss], kcos[:], ukp[:])
                    nc.scalar.dma_start(kT_sb[DH:P, ss], kT_sb[0:DH, ss])

                for sc in range(N_SC):
                    ss = slice(sc * SC, (sc + 1) * SC)
                    if sc == 0:
                        # prioritized first loads: kt 0-1 of everything first
                        # so the first matmul's inputs land ASAP
                        hs_t = hs_pool.tile([P, KT, SC], BF16, tag="hs",
                                            name="hs_0")
                        hsp_r = hsp_d[0].rearrange("p (kt s) -> p kt s", kt=KT)
                        for k0, k1 in ((0, 2), (2, 8), (8, KT)):
                            nc.sync.dma_start(wq_sb[:, k0:k1, :],
                                              wqp_r[:, k0:k1, :])
                            nc.sync.dma_start(wkv_sb[:, k0:k1, :],
                                              wkvp_r[:, k0:k1, :])
                            nc.sync.dma_start(hs_t[:, k0:k1, :],
                                              hsp_r[:, k0:k1, :])
                    else:
                        hs_t = hs_next
                    if sc + 1 < N_SC:
                        # prefetch the next chunk ahead of the constant loads
                        hs_next = hs_pool.tile([P, KT, SC], BF16, tag="hs",
                                               name=f"hs_{sc + 1}")
                        hsp_n = hsp_d[sc + 1].rearrange("p (kt s) -> p kt s",
                                                        kt=KT)
                        for k4 in range(2):
                            nc.sync.dma_start(hs_next[:, 8 * k4:8 * k4 + 8, :],
                                              hsp_n[:, 8 * k4:8 * k4 + 8, :])
                    if sc == 0:
                        # rope constants: must be emitted before this chunk's
                        # rope DVE ops (emission order defines dependencies)
                        nc.sync.dma_start(bv_sb[:], bv_d[:])
                        nc.sync.dma_start(cos_sb[:], cos_d[:])
                        nc.sync.dma_start(ssin_sb[:], ssin_d[:])
                        nc.sync.dma_start(v_aug[:, :, 0:DH], vpad_d[:])
                    if sc == 1:
                        # attention-only constants, queued behind the hot path
                        nc.sync.dma_start(tneg_sb[:], tneg_d[:])
                        nc.sync.dma_start(
                            ident2_sb[:],
                            ident2_d.rearrange("p (h n) -> p h n", h=2))
                        nc.sync.dma_start(bob_sb[:], bob_d[:])
                    if sc >= 1:
                        # spread the 8MB wo load across the projection phase
                        j0 = 5 * (sc - 1)
                        for jt in range(j0, min(j0 + 5, KT)):
                            nc.sync.dma_start(wo_sb[:, jt, :], wop_r[:, jt, :])

                    ps_q0 = proj_ps.tile([P, SC], F32, tag="proj")
                    ps_q1 = proj_ps.tile([P, SC], F32, tag="proj")
                    ps_kv = proj_ps.tile([P, SC], F32, tag="proj")
                    for kt in range(KT):
                        st = kt == 0
                        sp = kt == KT - 1
                        nc.tensor.matmul(ps_q0, wq_sb[:, kt, 0:P],
                                         hs_t[:, kt, :], start=st, stop=sp)
                        nc.tensor.matmul(ps_q1, wq_sb[:, kt, P:2 * P],
                                         hs_t[:, kt, :], start=st, stop=sp)
                        nc.tensor.matmul(ps_kv, wkv_sb[:, kt, :],
                                         hs_t[:, kt, :], start=st, stop=sp)

                    # v (+bias, cast bf16); transpose into v_aug via DMA
                    vT = rope_pool.tile([P, SC], BF16, tag="vT",
                                        name=f"vT_{sc}")
                    nc.vector.tensor_scalar_add(vT[DH:P, :], ps_kv[DH:P, :],
                                                bv_sb[DH:P, :])
                    emit_rope_q(0, ps_q0, ss, sc)
                    emit_rope_q(1, ps_q1, ss, sc)
                    emit_rope_k(ps_kv, ss, sc)
                    for k4 in range(4):
                        g = sc * (SC // P) + k4
                        nc.scalar.dma_start_transpose(
                            v_aug[:, g, DH:P],
                            vT[DH:P, k4 * P:(k4 + 1) * P])
                # remaining wo slice (jt 15)
                for jt in range(15, KT):
                    nc.sync.dma_start(wo_sb[:, jt, :], wop_r[:, jt, :])

            # ================= attention (lag-2 software pipeline) ========
            # i-chunk order [2,3,0,1]: pr=1 (chunks 2,3) a2a fires ~55% in,
            # pr=0 fires at the end and is overlapped by the pr=1 oproj.
            with tc.tile_pool(name="sc_ps", bufs=2, space="PSUM") as sc_ps_pool, \
                 tc.tile_pool(name="ot_ps", bufs=2, space="PSUM") as ot_ps_pool, \
                 tc.tile_pool(name="expa", bufs=4) as expa_pool, \
                 tc.tile_pool(name="norm", bufs=4) as norm_pool, \
                 tc.tile_pool(name="osb", bufs=2) as o_pool, \
                 tc.tile_pool(name="otsb", bufs=4) as ot_sb_pool:
                groups = [(ic, hp) for ic in (2, 3, 0, 1) for hp in range(2)]
                ex_of = {}     # (ic, hp, jt) -> (ex tile, fsl)
                ot_of = {}     # (ic, hp) -> ot tile
                onrm_of = {}   # (ic, hm) -> coalesced store tile

                def emit_S(ic, hp, jt):
                    r = jt - 4 * ic
                    f0 = max(r, 0) * P
                    fsl = slice(f0, SC)
                    jsl = slice(jt * P, (jt + 1) * P)
                    qisl = slice(ic * SC + f0, (ic + 1) * SC)
                    sc_t = sc_ps_pool.tile([P, 2, SC], F32, tag="sc",
                                           name=f"sc_{ic}_{hp}_{jt}")
                    nc.tensor.matmul(sc_t[:, 0, fsl], kT_sb[0:DH, jsl],
                                     qT_sb[0:DH, hp, qisl],
                                     start=True, stop=(r < 0))
                    nc.tensor.matmul(sc_t[:, 1, fsl], kT_sb[DH:P, jsl],
                                     qT_sb[DH:P, hp, qisl],
                                     start=True, stop=(r < 0))
                    if r >= 0:
                        rsl = slice(f0, f0 + P)
                        nc.tensor.matmul(sc_t[:, :, rsl], tneg_sb[:],
                                         ident2_sb[:], start=False, stop=True,
                                         skip_group_check=True)
                    ex = expa_pool.tile([P, 2, SC], BF16, tag="ex",
                                        name=f"ex_{ic}_{hp}_{jt}")
                    nc.scalar.activation(ex[:, :, fsl], sc_t[:, :, fsl],
                                         Exp, scale=float(SCALE))
                    ex_of[(ic, hp, jt)] = (ex, fsl)

                def emit_A(ic, hp, jt):
                    n_jt = 4 * (ic + 1)
                    if jt == 0:
                        ot_of[(ic, hp)] = ot_ps_pool.tile(
                            [P, 2, SC], F32, tag="ot", name=f"ot_{ic}_{hp}")
                    ot = ot_of[(ic, hp)]
                    ex, fsl = ex_of.pop((ic, hp, jt))
                    st = jt == 0
                    sp = jt == n_jt - 1
                    nc.tensor.matmul(ot[:, 0, fsl], v_aug[:, jt, :],
                                     ex[:, 0, fsl], start=st, stop=sp,
                                     skip_group_check=True)
                    nc.tensor.matmul(ot[:, 1, fsl], v_aug[:, jt, :],
                                     ex[:, 1, fsl], start=st, stop=sp,
                                     skip_group_check=True)

                def emit_norm(ic, hp, _half):
                    # ot rows: 0 = sum(exp), 64:128 = v-dims
                    ot = ot_of[(ic, hp)]
                    pr, q = ic // 2, ic % 2
                    if hp == 0:
                        for hm in range(2):
                            onrm_of[(ic, hm)] = ot_sb_pool.tile(
                                [P, N_CORES, 2, DH], BF16, tag=f"otsb{hm}",
                                name=f"onrm_{ic}_{hm}")
                    bcs = []
                    for half in range(2):
                        hl = 2 * hp + half
                        recip = norm_pool.tile([1, SC], F32, tag="recip",
                                               name=f"recip_{ic}_{hl}")
                        nc.vector.reciprocal_approx_fast(recip[:],
                                                         ot[0:1, half, :])
                        recipb = norm_pool.tile([1, SC], BF16, tag="recipb",
                                                name=f"recipb_{ic}_{hl}")
                        nc.vector.tensor_copy(recipb[:], recip[:])
                        # replicate across partitions on the (idle) gpsimd
                        # engine; both halves' broadcasts overlap the DVE work
                        bc = norm_pool.tile([P, SC], BF16, tag="bcsb",
                                            name=f"bc_{ic}_{hl}")
                        nc.gpsimd.partition_broadcast(bc[:], recipb[:])
                        bcs.append(bc)
                    for half in range(2):
                        hm = half
                        onrm = onrm_of[(ic, hm)]
                        nc.vector.tensor_mul(
                            onrm[DH:P, :, hp, :],
                            ot[DH:P, half, :].rearrange(
                                "d (e s) -> d e s", e=N_CORES),
                            bcs[half][DH:P, :].rearrange(
                                "d (e s) -> d e s", e=N_CORES))
                    if hp == 1:
                        # both head-pairs done: store (DMA APs are <=3 dims,
                        # so one store per (d-half, head-pair))
                        for hm in range(2):
                            onrm = onrm_of.pop((ic, hm))
                            for t in range(2):
                                nc.sync.dma_start(
                                    a2a_in[pr][:, hm * DH:(hm + 1) * DH,
                                               t, q, :]
                                    .rearrange("e d s -> d e s"),
                                    onrm[DH:P, :, t, :])

                def emit_a2a(pr):
                    # flat per-dest APs: large contiguous runs for the cc DMAs
                    nc.gpsimd.collective_compute(
                        "AllToAll", mybir.AluOpType.bypass,
                        replica_groups=grp,
                        ins=[a2a_in[pr].rearrange("e p t q s -> e (p t q s)")],
                        outs=[a2a_out[pr].rearrange("e p t q s -> e (p t q s)")])
                    # queue the oproj input load now: one DMA, 512B runs
                    o_t = o_pool.tile([P, N_CORES, 2, 2, DH], BF16, tag="o",
                                      name=f"o_{pr}")
                    nc.sync.dma_start(
                        o_t[:],
                        a2a_out[pr].rearrange("e p t q s -> p e t q s"))
                    o_ts[pr] = o_t

                o_ts = {}
                S_items = []
                A_items = []
                for ic, hp in groups:
                    n_jt = 4 * (ic + 1)
                    for jt in range(n_jt):
                        S_items.append(("S", ic, hp, jt))
                        A_items.append(("A", ic, hp, jt))
                    A_items.append(("N", ic, hp, 0))
                    if hp == 1 and ic in (3, 1):
                        A_items.append(("C", 1 if ic == 3 else 0, 0, 0))

                def run(item):
                    kind, a, b, c = item
                    if kind == "S":
                        emit_S(a, b, c)
                    elif kind == "A":
                        emit_A(a, b, c)
                    elif kind == "N":
                        emit_norm(a, b, c)
                    else:
                        emit_a2a(a)

                si = 0
                for _ in range(2):
                    run(S_items[si]); si += 1
                for ai in range(len(A_items)):
                    run(A_items[ai])
                    if si < len(S_items):
                        run(S_items[si]); si += 1

            # ================= output projection =================
            # pr=1 (chunks 2,3) first — its a2a landed mid-attention; the
            # pr=0 a2a flies underneath it. jt tile: source core e = jt//2,
            # head-pair t = jt%2.
            with tc.tile_pool(name="out_ps", bufs=4, space="PSUM") as out_ps_pool, \
                 tc.tile_pool(name="outsb", bufs=4) as out_sb_pool:
                for pr in (1, 0):
                    o_t = o_ts[pr]
                    op_ps = [out_ps_pool.tile([P, SC], F32, tag="op",
                                              name=f"op_{pr}_{n4}")
                             for n4 in range(4)]
                    for jt in range(KT):
                        e, t = jt // 2, jt % 2
                        for n4 in range(4):
                            nc.tensor.matmul(
                                op_ps[n4], o_t[:, e, t, :, :],
                                wo_sb[:, jt, n4 * SC:(n4 + 1) * SC],
                                start=(jt == 0), stop=(jt == KT - 1))
                    for n4 in range(4):
                        nsl = slice(n4 * SC, (n4 + 1) * SC)
                        out_sb = out_sb_pool.tile([P, SC], F32, tag="outsb")
                        nc.vector.tensor_add(out_sb[:], op_ps[n4],
                                             bob_sb[:, nsl])
                        nc.sync.dma_start(out_d[pr * P:(pr + 1) * P, nsl],
                                          out_sb[:])

    nc.compile()
    return nc


_cached_nc = None


def kernel(hidden_states, attention_mask, cos, sin, Wq, Wk, Wv, bv, Wo, bo):
    global _cached_nc, last_results
    hidden_states = np.asarray(hidden_states, dtype=np.float32)
    attention_mask = np.asarray(attention_mask)
    if not np.all(attention_mask == 1):
        raise NotImplementedError("kernel assumes an all-ones attention_mask")
    cos = np.asarray(cos, dtype=np.float32)
    sin = np.asarray(sin, dtype=np.float32)
    Wq = np.asarray(Wq, dtype=np.float32)
    Wk = np.asarray(Wk, dtype=np.float32)
    Wv = np.asarray(Wv, dtype=np.float32)
    bv = np.asarray(bv, dtype=np.float32)
    Wo = np.asarray(Wo, dtype=np.float32)
    bo = np.asarray(bo, dtype=np.float32)
    bf = ml_dtypes.bfloat16

    hsT = hidden_states[0].T.astype(bf)                           # [HID, S]
    hsp = np.ascontiguousarray(
        hsT.reshape(KT, P, N_SC, SC).transpose(2, 1, 0, 3).reshape(
            N_SC, P, KT * SC))
    cosT = np.ascontiguousarray(cos[0].T)                         # [DH, S]
    sinT = np.ascontiguousarray(sin[0].T)
    cos2 = np.concatenate([cosT, cosT], axis=0).astype(bf)        # [128, S]
    # sign-and-partner folded sin: u = q*ssin, then a 32-row half-swap of u
    # gives rot_half(q)*sin.  ssin[x] = sin[x+32] for x<32, -sin[x-32] else.
    ssinT = np.concatenate([sinT[HH:DH], -sinT[0:HH]], axis=0)    # [DH, S]
    ssin2 = np.concatenate([ssinT, ssinT], axis=0).astype(bf)     # [128, S]

    # causal mask accumulate: psum[j,h,i'] += tneg[i',j] = -1e30 if j > i'
    kk = np.arange(P)[:, None]
    mm = np.arange(P)[None, :]
    tneg = np.where(mm > kk, np.float32(-1e30), np.float32(0)).astype(bf)
    ident2 = np.zeros((P, 2, P), dtype=bf)
    for h in range(2):
        ident2[:, h, :] = np.eye(P, dtype=bf)
    ident2 = np.ascontiguousarray(ident2.reshape(P, 2 * P))

    vpad = np.zeros((P, ST, DH), dtype=bf)
    vpad[:, :, 0] = 1.0
    woT = Wo.T.astype(bf)                                         # [NH*DH, HID]
    wop = np.ascontiguousarray(
        woT.reshape(KT, P, HID).transpose(1, 0, 2).reshape(P, KT * HID))
    bob = np.broadcast_to(bo.reshape(1, HID), (P, HID)).astype(bf)
    bob = np.ascontiguousarray(bob)

    in_maps = []
    for c in range(N_CORES):
        wqT_c = Wq[c * NH_C * DH:(c + 1) * NH_C * DH].T.astype(bf)
        wqp_c = np.ascontiguousarray(
            wqT_c.reshape(KT, P, NH_C * DH).transpose(1, 0, 2).reshape(
                P, KT * NH_C * DH))
        wkv_c = np.concatenate([Wk[c * DH:(c + 1) * DH],
                                Wv[c * DH:(c + 1) * DH]], axis=0)
        wkvT_c = wkv_c.T.astype(bf)
        wkvp_c = np.ascontiguousarray(
            wkvT_c.reshape(KT, P, 2 * DH).transpose(1, 0, 2).reshape(
                P, KT * 2 * DH))
        bv_c = np.zeros((P, 1), dtype=np.float32)
        bv_c[DH:, 0] = bv[c * DH:(c + 1) * DH]
        in_maps.append({
            "hsp": hsp, "wqp": wqp_c, "wkvp": wkvp_c, "bv": bv_c,
            "cos2": cos2, "ssin2": ssin2, "tneg": tneg,
            "ident2": ident2, "vpad": vpad, "wop": wop, "bob": bob,
        })

    if _cached_nc is None:
        _cached_nc = _build()
    res = run_bass_kernel_spmd(_cached_nc, in_maps, list(range(N_CORES)))
    last_results = res
    if res.exec_time_ns is not None:
        print(f"HW exec time: {res.exec_time_ns} ns")

    # core e's out rows: 128*pr + 64*q + r -> global 512*(2*pr+q) + 64*e + r
    res_all = np.stack([res.results[c]["out"] for c in range(N_CORES)])
    out = res_all.reshape(N_CORES, 2, 2, DH, HID).transpose(1, 2, 0, 3, 4)
    out = np.ascontiguousarray(out.reshape(1, S, HID))
    return out.astype(np.float32)


# revision 26
# speedup vs baseline: 1.2004x; 1.2004x over previous
"""GQA attention block (B=1, S=2048, HID=2048, NH=32, NKV=8, DH=64) on 8 trn2
NeuronCores — v4.

Sharding: tensor-parallel over heads (core c: query heads [4c,4c+4), KV head
c), then a sequence re-shard for the output projection.

v4 over v3:
- startup: the first projection matmul's inputs (wq/wkv/hs for kt 0-1) load
  first as small DMAs, so compute starts ~7us earlier.
- RoPE rotate-half is a partition-permute DMA of u = q*ssin (sign-and-partner
  folded into the ssin table) instead of a PE matmul; V transposes use the
  DMA transpose engine. cos/ssin ship as bf16 [128,S]. The PE only does
  projections, scores (+causal diag), AV, and the output projection.
- output-projection bias rides the PSUM->SBUF eviction as a DVE add with a
  host-broadcast bias table (no ones/bias matmuls).
- attention runs i-chunks in order [2,3,0,1]: the {2,3} AllToAll fires at
  ~55% of attention and lands mid-phase; the {0,1} AllToAll flies while the
  {2,3} output projection runs, so no collective is exposed.
- a2a stores coalesce both head-pairs per (ic, d-half) with 256B dram runs;
  the a2a output reloads as one DMA per pair with 512B runs.
- wo streams in 2MB slices during the projection phase to smooth DMA power.

All matmul inputs are bf16 (fp32 PSUM accumulation). attention_mask must be
all-ones (the problem spec fill); only the causal mask is applied.
"""

import sys

sys.path.insert(0, "/opt/trn_rl_repo")

import numpy as np
import ml_dtypes

import concourse.bacc as bacc
import concourse.mybir as mybir
import concourse.tile as tile
from concourse.bass_utils import run_bass_kernel_spmd

F32 = mybir.dt.float32
BF16 = mybir.dt.bfloat16
Exp = mybir.ActivationFunctionType.Exp

N_CORES = 8
S = 2048
HID = 2048
NH, NKV, DH = 32, 8, 64
NH_C = NH // N_CORES          # 4 query heads per core
P = 128
SC = 512                      # i-chunk (attention free dim)
N_SC = S // SC                # 4
KT = HID // P                 # 16 contraction tiles
ST = S // P                   # 16 key tiles of 128
SCALE = 1.0 / np.sqrt(DH)
SSH = S // N_CORES            # 256 output rows per core
HH = DH // 2                  # 32, rotate-half block

last_results = None


def _build():
    nc = bacc.Bacc("TRN2", target_bir_lowering=False, debug=False,
                   num_devices=N_CORES)

    # ---- kernel I/O ----
    hsp_d = nc.dram_tensor("hsp", [N_SC, P, KT * SC], BF16, kind="ExternalInput")
    wqp_d = nc.dram_tensor("wqp", [P, KT * NH_C * DH], BF16, kind="ExternalInput")
    wkvp_d = nc.dram_tensor("wkvp", [P, KT * 2 * DH], BF16, kind="ExternalInput")
    bv_d = nc.dram_tensor("bv", [P, 1], F32, kind="ExternalInput")
    cos_d = nc.dram_tensor("cos2", [P, S], BF16, kind="ExternalInput")
    sin_d = nc.dram_tensor("sin2", [P, S], BF16, kind="ExternalInput")
    rotw_d = nc.dram_tensor("rotw", [P, P], BF16, kind="ExternalInput")
    identj_d = nc.dram_tensor("identj", [P, DH], BF16, kind="ExternalInput")
    tneg_d = nc.dram_tensor("tneg", [P, P], BF16, kind="ExternalInput")
    ident2_d = nc.dram_tensor("ident2", [P, 2 * P], BF16, kind="ExternalInput")
    vpad_d = nc.dram_tensor("vpad", [P, ST, DH], BF16, kind="ExternalInput")
    wop_d = nc.dram_tensor("wop", [P, KT * HID], BF16, kind="ExternalInput")
    bob_d = nc.dram_tensor("bob", [P, HID], BF16, kind="ExternalInput")
    out_d = nc.dram_tensor("out", [SSH, HID], F32, kind="ExternalOutput")

    # sequence re-shard: two AllToAlls over [e][p][t][q][s] blocks.
    # pr=1 carries i-chunks {2,3} (fires mid-attention), pr=0 carries {0,1}
    # (fires last, overlapped by the pr=1 output projection).
    a2a_in = [nc.dram_tensor(f"a2a_in{i}", [N_CORES, P, 2, 2, DH], BF16)
              for i in range(2)]
    a2a_out = [nc.dram_tensor(f"a2a_out{i}", [N_CORES, P, 2, 2, DH], BF16)
               for i in range(2)]
    warm_in = nc.dram_tensor("warm_in", [N_CORES, 1, 2], BF16)
    warm_out = nc.dram_tensor("warm_out", [N_CORES, 1, 2], BF16)
    grp = [list(range(N_CORES))]

    with tile.TileContext(nc) as tc:
        # fire the collective rendezvous barrier immediately so it overlaps
        # the projection phase instead of the first real AllToAll
        nc.gpsimd.collective_compute(
            "AllToAll", mybir.AluOpType.bypass, replica_groups=grp,
            ins=[warm_in[:]], outs=[warm_out[:]])

        with tc.tile_pool(name="persist", bufs=1) as persist:
            # dummy broadcast so the gpsimd extended library loads during the
            # initial DMA phase, not at the first normalization
            warmbc = persist.tile([P, 8], BF16)
            nc.gpsimd.partition_broadcast(warmbc[:], warmbc[0:1, :])
            wq_sb = persist.tile([P, KT, NH_C * DH], BF16)
            wkv_sb = persist.tile([P, KT, 2 * DH], BF16)
            cos_sb = persist.tile([P, S], BF16)
            sin_sb = persist.tile([P, S], BF16)
            rotw_sb = persist.tile([P, P], BF16)
            identj_sb = persist.tile([P, DH], BF16)
            tneg_sb = persist.tile([P, P], BF16)
            ident2_sb = persist.tile([P, 2, P], BF16)
            bv_sb = persist.tile([P, 1], F32)
            bob_sb = persist.tile([P, HID], BF16)

            # ---- persistent activations ----
            qT_sb = persist.tile([P, 2, S], BF16)      # 4 heads, 2 per tile
            kT_sb = persist.tile([P, S], BF16)         # rows 0:64 kT, 64:128 dup
            v_aug = persist.tile([P, ST, P], BF16)     # [s, s-tile, ones+pad+v]
            wo_sb = persist.tile([P, KT, HID], BF16)

            wqp_r = wqp_d.rearrange("p (kt m) -> p kt m", kt=KT)
            wkvp_r = wkvp_d.rearrange("p (kt m) -> p kt m", kt=KT)
            wop_r = wop_d.rearrange("p (jt n) -> p jt n", jt=KT)

            # ================= QKV projection + RoPE =================
            # chunk k's rope/transpose PE ops are deferred and interleaved
            # into chunk k+1's projection matmul stream
            with tc.tile_pool(name="hs", bufs=2) as hs_pool, \
                 tc.tile_pool(name="proj_ps", bufs=6, space="PSUM") as proj_ps, \
                 tc.tile_pool(name="tp_ps", bufs=1, space="PSUM") as tp_ps, \
                 tc.tile_pool(name="rot_ps", bufs=1, space="PSUM") as rot_ps, \
                 tc.tile_pool(name="rope", bufs=4) as rope_pool:
                deferred = []

                def make_rot_q(m, ps_q, ss, sc):
                    qcos = rope_pool.tile([P, SC], F32, tag="qcos",
                                          name=f"qcos_{sc}_{m}")
                    nc.vector.tensor_mul(qcos[:], ps_q[:], cos_sb[:, ss])
                    qraw = rope_pool.tile([P, SC], BF16, tag="qraw",
                                          name=f"qraw_{sc}_{m}")
                    nc.vector.tensor_copy(qraw[:], ps_q[:])

                    def emit():
                        rot = rot_ps.tile([P, SC], F32, tag="rot",
                                          name=f"rot_{sc}_{m}")
                        nc.tensor.matmul(rot, rotw_sb[:], qraw[:],
                                         start=True, stop=True)
                        qsin = rope_pool.tile([P, SC], F32, tag="qsin",
                                              name=f"qsin_{sc}_{m}")
                        nc.vector.tensor_mul(qsin[:], rot[:], sin_sb[:, ss])
                        nc.vector.tensor_add(qT_sb[:, m, ss], qcos[:], qsin[:])
                    return emit

                def make_rot_k(ps_kv, ss, sc):
                    kcos = rope_pool.tile([DH, SC], F32, tag="kcos",
                                          name=f"kcos_{sc}")
                    nc.vector.tensor_mul(kcos[:], ps_kv[0:DH, :],
                                         cos_sb[0:DH, ss])
                    kraw = rope_pool.tile([DH, SC], BF16, tag="kraw",
                                          name=f"kraw_{sc}")
                    nc.vector.tensor_copy(kraw[:], ps_kv[0:DH, :])

                    def emit():
                        krot = rot_ps.tile([DH, SC], F32, tag="rot",
                                           name=f"krot_{sc}")
                        nc.tensor.matmul(krot, rotw_sb[0:DH, 0:DH], kraw[:],
                                         start=True, stop=True)
                        ksin = rope_pool.tile([DH, SC], F32, tag="ksin",
                                              name=f"ksin_{sc}")
                        nc.vector.tensor_mul(ksin[:], krot[:], sin_sb[0:DH, ss])
                        nc.vector.tensor_add(kT_sb[0:DH, ss], kcos[:], ksin[:])
                        nc.sync.dma_start(kT_sb[DH:P, ss], kT_sb[0:DH, ss])
                    return emit

                def make_tp(vT, k4, sc):
                    def emit():
                        g = sc * (SC // P) + k4
                        tp = tp_ps.tile([P, DH], BF16, tag="tp",
                                        name=f"tp_{sc}_{k4}")
                        nc.tensor.transpose(tp, vT[DH:P, k4 * P:(k4 + 1) * P],
                                            identj_sb[DH:P, :])
                        nc.vector.tensor_copy(v_aug[:, g, DH:P], tp[:])
                    return emit

                for sc in range(N_SC):
                    ss = slice(sc * SC, (sc + 1) * SC)
                    if sc == 0:
                        # prioritized first loads: kt 0-1 of everything first
                        # so the first matmul's inputs land ASAP
                        hs_t = hs_pool.tile([P, KT, SC], BF16, tag="hs",
                                            name="hs_0")
                        hsp_r = hsp_d[0].rearrange("p (kt s) -> p kt s", kt=KT)
                        for k0, k1 in ((0, 2), (2, 8), (8, KT)):
                            nc.sync.dma_start(wq_sb[:, k0:k1, :],
                                              wqp_r[:, k0:k1, :])
                            nc.sync.dma_start(wkv_sb[:, k0:k1, :],
                                              wkvp_r[:, k0:k1, :])
                            nc.sync.dma_start(hs_t[:, k0:k1, :],
                                              hsp_r[:, k0:k1, :])
                    else:
                        hs_t = hs_next
                    if sc + 1 < N_SC:
                        # prefetch the next chunk ahead of the constant loads
                        hs_next = hs_pool.tile([P, KT, SC], BF16, tag="hs",
                                               name=f"hs_{sc + 1}")
                        hsp_n = hsp_d[sc + 1].rearrange("p (kt s) -> p kt s",
                                                        kt=KT)
                        for k4 in range(2):
                            nc.sync.dma_start(hs_next[:, 8 * k4:8 * k4 + 8, :],
                                              hsp_n[:, 8 * k4:8 * k4 + 8, :])
                    if sc == 0:
                        # rope constants: must be emitted before this chunk's
                        # rope DVE ops (emission order defines dependencies)
                        nc.sync.dma_start(rotw_sb[:], rotw_d[:])
                        nc.sync.dma_start(bv_sb[:], bv_d[:])
                        nc.sync.dma_start(cos_sb[:], cos_d[:])
                        nc.sync.dma_start(sin_sb[:], sin_d[:])
                    if sc == 1:
                        # attention-only constants, queued behind the hot path
                        nc.sync.dma_start(tneg_sb[:], tneg_d[:])
                        nc.sync.dma_start(
                            ident2_sb[:],
                            ident2_d.rearrange("p (h n) -> p h n", h=2))
                        nc.sync.dma_start(bob_sb[:], bob_d[:])
                    if sc >= 1:
                        # spread the 8MB wo load across the projection phase
                        j0 = 5 * (sc - 1)
                        for jt in range(j0, min(j0 + 5, KT)):
                            nc.sync.dma_start(wo_sb[:, jt, :], wop_r[:, jt, :])

                    ps_q0 = proj_ps.tile([P, SC], F32, tag="proj")
                    ps_q1 = proj_ps.tile([P, SC], F32, tag="proj")
                    ps_kv = proj_ps.tile([P, SC], F32, tag="proj")
                    for kt in range(KT):
                        st = kt == 0
                        sp = kt == KT - 1
                        nc.tensor.matmul(ps_q0, wq_sb[:, kt, 0:P],
                                         hs_t[:, kt, :], start=st, stop=sp)
                        nc.tensor.matmul(ps_q1, wq_sb[:, kt, P:2 * P],
                                         hs_t[:, kt, :], start=st, stop=sp)
                        nc.tensor.matmul(ps_kv, wkv_sb[:, kt, :],
                                         hs_t[:, kt, :], start=st, stop=sp)
                        if kt >= 9 and deferred:
                            # pop late in the chunk so the DVE pre-ops of the
                            # previous chunk's rope are finished by then
                            deferred.pop(0)()

                    if sc == 0:
                        # needed by chunk-0 rope/transpose pops in chunk 1
                        nc.sync.dma_start(identj_sb[:], identj_d[:])
                        nc.sync.dma_start(v_aug[:, :, 0:DH], vpad_d[:])

                    # v (+bias, cast bf16) at rows 64:128 of the kv psum
                    vT = rope_pool.tile([P, SC], BF16, tag="vT",
                                        name=f"vT_{sc}")
                    nc.vector.tensor_scalar_add(vT[DH:P, :], ps_kv[DH:P, :],
                                                bv_sb[DH:P, :])
                    deferred.extend([
                        make_rot_q(0, ps_q0, ss, sc),
                        make_rot_q(1, ps_q1, ss, sc),
                        make_rot_k(ps_kv, ss, sc),
                        make_tp(vT, 0, sc), make_tp(vT, 1, sc),
                        make_tp(vT, 2, sc), make_tp(vT, 3, sc),
                    ])
                for f in deferred:
                    f()
                # remaining wo slice (jt 15)
                for jt in range(15, KT):
                    nc.sync.dma_start(wo_sb[:, jt, :], wop_r[:, jt, :])

            # ================= attention (lag-2 software pipeline) ========
            # i-chunk order [2,3,0,1]: pr=1 (chunks 2,3) a2a fires ~55% in,
            # pr=0 fires at the end and is overlapped by the pr=1 oproj.
            with tc.tile_pool(name="sc_ps", bufs=2, space="PSUM") as sc_ps_pool, \
                 tc.tile_pool(name="ot_ps", bufs=2, space="PSUM") as ot_ps_pool, \
                 tc.tile_pool(name="expa", bufs=4) as expa_pool, \
                 tc.tile_pool(name="norm", bufs=4) as norm_pool, \
                 tc.tile_pool(name="osb", bufs=2) as o_pool, \
                 tc.tile_pool(name="otsb", bufs=4) as ot_sb_pool:
                groups = [(ic, hp) for ic in (2, 3, 0, 1) for hp in range(2)]
                ex_of = {}     # (ic, hp, jt) -> (ex tile, fsl)
                ot_of = {}     # (ic, hp) -> ot tile
                onrm_of = {}   # (ic, hm) -> coalesced store tile

                def emit_S(ic, hp, jt):
                    r = jt - 4 * ic
                    f0 = max(r, 0) * P
                    fsl = slice(f0, SC)
                    jsl = slice(jt * P, (jt + 1) * P)
                    qisl = slice(ic * SC + f0, (ic + 1) * SC)
                    sc_t = sc_ps_pool.tile([P, 2, SC], F32, tag="sc",
                                           name=f"sc_{ic}_{hp}_{jt}")
                    nc.tensor.matmul(sc_t[:, 0, fsl], kT_sb[0:DH, jsl],
                                     qT_sb[0:DH, hp, qisl],
                                     start=True, stop=(r < 0))
                    nc.tensor.matmul(sc_t[:, 1, fsl], kT_sb[DH:P, jsl],
                                     qT_sb[DH:P, hp, qisl],
                                     start=True, stop=(r < 0))
                    if r >= 0:
                        rsl = slice(f0, f0 + P)
                        nc.tensor.matmul(sc_t[:, :, rsl], tneg_sb[:],
                                         ident2_sb[:], start=False, stop=True,
                                         skip_group_check=True)
                    ex = expa_pool.tile([P, 2, SC], BF16, tag="ex",
                                        name=f"ex_{ic}_{hp}_{jt}")
                    nc.scalar.activation(ex[:, :, fsl], sc_t[:, :, fsl],
                                         Exp, scale=float(SCALE))
                    ex_of[(ic, hp, jt)] = (ex, fsl)

                def emit_A(ic, hp, jt):
                    n_jt = 4 * (ic + 1)
                    if jt == 0:
                        ot_of[(ic, hp)] = ot_ps_pool.tile(
                            [P, 2, SC], F32, tag="ot", name=f"ot_{ic}_{hp}")
                    ot = ot_of[(ic, hp)]
                    ex, fsl = ex_of.pop((ic, hp, jt))
                    st = jt == 0
                    sp = jt == n_jt - 1
                    nc.tensor.matmul(ot[:, 0, fsl], v_aug[:, jt, :],
                                     ex[:, 0, fsl], start=st, stop=sp,
                                     skip_group_check=True)
                    nc.tensor.matmul(ot[:, 1, fsl], v_aug[:, jt, :],
                                     ex[:, 1, fsl], start=st, stop=sp,
                                     skip_group_check=True)

                def emit_norm(ic, hp, _half):
                    # ot rows: 0 = sum(exp), 64:128 = v-dims
                    ot = ot_of[(ic, hp)]
                    pr, q = ic // 2, ic % 2
                    if hp == 0:
                        for hm in range(2):
                            onrm_of[(ic, hm)] = ot_sb_pool.tile(
                                [P, N_CORES, 2, DH], BF16, tag=f"otsb{hm}",
                                name=f"onrm_{ic}_{hm}")
                    bcs = []
                    for half in range(2):
                        hl = 2 * hp + half
                        recip = norm_pool.tile([1, SC], F32, tag="recip",
                                               name=f"recip_{ic}_{hl}")
                        nc.vector.reciprocal_approx_fast(recip[:],
                                                         ot[0:1, half, :])
                        recipb = norm_pool.tile([1, SC], BF16, tag="recipb",
                                                name=f"recipb_{ic}_{hl}")
                        nc.vector.tensor_copy(recipb[:], recip[:])
                        # replicate across partitions on the (idle) gpsimd
                        # engine; both halves' broadcasts overlap the DVE work
                        bc = norm_pool.tile([P, SC], BF16, tag="bcsb",
                                            name=f"bc_{ic}_{hl}")
                        nc.gpsimd.partition_broadcast(bc[:], recipb[:])
                        bcs.append(bc)
                    for half in range(2):
                        hm = half
                        onrm = onrm_of[(ic, hm)]
                        nc.vector.tensor_mul(
                            onrm[DH:P, :, hp, :],
                            ot[DH:P, half, :].rearrange(
                                "d (e s) -> d e s", e=N_CORES),
                            bcs[half][DH:P, :].rearrange(
                                "d (e s) -> d e s", e=N_CORES))
                    if hp == 1:
                        # both head-pairs done: store (DMA APs are <=3 dims,
                        # so one store per (d-half, head-pair))
                        for hm in range(2):
                            onrm = onrm_of.pop((ic, hm))
                            for t in range(2):
                                nc.sync.dma_start(
                                    a2a_in[pr][:, hm * DH:(hm + 1) * DH,
                                               t, q, :]
                                    .rearrange("e d s -> d e s"),
                                    onrm[DH:P, :, t, :])

                def emit_a2a(pr):
                    # flat per-dest APs: large contiguous runs for the cc DMAs
                    nc.gpsimd.collective_compute(
                        "AllToAll", mybir.AluOpType.bypass,
                        replica_groups=grp,
                        ins=[a2a_in[pr].rearrange("e p t q s -> e (p t q s)")],
                        outs=[a2a_out[pr].rearrange("e p t q s -> e (p t q s)")])
                    # queue the oproj input load now: one DMA, 512B runs
                    o_t = o_pool.tile([P, N_CORES, 2, 2, DH], BF16, tag="o",
                                      name=f"o_{pr}")
                    nc.sync.dma_start(
                        o_t[:],
                        a2a_out[pr].rearrange("e p t q s -> p e t q s"))
                    o_ts[pr] = o_t

                o_ts = {}
                S_items = []
                A_items = []
                for ic, hp in groups:
                    n_jt = 4 * (ic + 1)
                    for jt in range(n_jt):
                        S_items.append(("S", ic, hp, jt))
                        A_items.append(("A", ic, hp, jt))
                    A_items.append(("N", ic, hp, 0))
                    if hp == 1 and ic in (3, 1):
                        A_items.append(("C", 1 if ic == 3 else 0, 0, 0))

                def run(item):
                    kind, a, b, c = item
                    if kind == "S":
                        emit_S(a, b, c)
                    elif kind == "A":
                        emit_A(a, b, c)
                    elif kind == "N":
                        emit_norm(a, b, c)
                    else:
                        emit_a2a(a)

                si = 0
                for _ in range(2):
                    run(S_items[si]); si += 1
                for ai in range(len(A_items)):
                    run(A_items[ai])
                    if si < len(S_items):
                        run(S_items[si]); si += 1

            # ================= output projection =================
            # pr=1 (chunks 2,3) first — its a2a landed mid-attention; the
            # pr=0 a2a flies underneath it. jt tile: source core e = jt//2,
            # head-pair t = jt%2.
            with tc.tile_pool(name="out_ps", bufs=4, space="PSUM") as out_ps_pool, \
                 tc.tile_pool(name="outsb", bufs=4) as out_sb_pool:
                for pr in (1, 0):
                    o_t = o_ts[pr]
                    op_ps = [out_ps_pool.tile([P, SC], F32, tag="op",
                                              name=f"op_{pr}_{n4}")
                             for n4 in range(4)]
                    for jt in range(KT):
                        e, t = jt // 2, jt % 2
                        for n4 in range(4):
                            nc.tensor.matmul(
                                op_ps[n4], o_t[:, e, t, :, :],
                                wo_sb[:, jt, n4 * SC:(n4 + 1) * SC],
                                start=(jt == 0), stop=(jt == KT - 1))
                    for n4 in range(4):
                        nsl = slice(n4 * SC, (n4 + 1) * SC)
                        out_sb = out_sb_pool.tile([P, SC], F32, tag="outsb")
                        nc.vector.tensor_add(out_sb[:], op_ps[n4],
                                             bob_sb[:, nsl])
                        nc.sync.dma_start(out_d[pr * P:(pr + 1) * P, nsl],
                                          out_sb[:])

    nc.compile()
    return nc


_cached_nc = None


def kernel(hidden_states, attention_mask, cos, sin, Wq, Wk, Wv, bv, Wo, bo):
    global _cached_nc, last_results
    hidden_states = np.asarray(hidden_states, dtype=np.float32)
    attention_mask = np.asarray(attention_mask)
    if not np.all(attention_mask == 1):
        raise NotImplementedError("kernel assumes an all-ones attention_mask")
    cos = np.asarray(cos, dtype=np.float32)
    sin = np.asarray(sin, dtype=np.float32)
    Wq = np.asarray(Wq, dtype=np.float32)
    Wk = np.asarray(Wk, dtype=np.float32)
    Wv = np.asarray(Wv, dtype=np.float32)
    bv = np.asarray(bv, dtype=np.float32)
    Wo = np.asarray(Wo, dtype=np.float32)
    bo = np.asarray(bo, dtype=np.float32)
    bf = ml_dtypes.bfloat16

    hsT = hidden_states[0].T.astype(bf)                           # [HID, S]
    hsp = np.ascontiguousarray(
        hsT.reshape(KT, P, N_SC, SC).transpose(2, 1, 0, 3).reshape(
            N_SC, P, KT * SC))
    cosT = np.ascontiguousarray(cos[0].T)                         # [DH, S]
    sinT = np.ascontiguousarray(sin[0].T)
    cos2 = np.concatenate([cosT, cosT], axis=0).astype(bf)        # [128, S]
    sin2 = np.concatenate([sinT, sinT], axis=0).astype(bf)        # [128, S]

    # rotate-half as a matmul: rot[d] = sign(d) * q[(d+32) % 64], per 64-block
    rotw = np.zeros((P, P), dtype=np.float32)
    for blk in (0, DH):
        for dd in range(DH):
            partner = (dd + HH) % DH
            sign = -1.0 if dd < HH else 1.0
            rotw[blk + partner, blk + dd] = sign
    rotw = rotw.astype(bf)
    identj = np.zeros((P, DH), dtype=bf)
    identj[DH:, :] = np.eye(DH, dtype=bf)

    # causal mask accumulate: psum[j,h,i'] += tneg[i',j] = -1e30 if j > i'
    kk = np.arange(P)[:, None]
    mm = np.arange(P)[None, :]
    tneg = np.where(mm > kk, np.float32(-1e30), np.float32(0)).astype(bf)
    ident2 = np.zeros((P, 2, P), dtype=bf)
    for h in range(2):
        ident2[:, h, :] = np.eye(P, dtype=bf)
    ident2 = np.ascontiguousarray(ident2.reshape(P, 2 * P))

    vpad = np.zeros((P, ST, DH), dtype=bf)
    vpad[:, :, 0] = 1.0
    woT = Wo.T.astype(bf)                                         # [NH*DH, HID]
    wop = np.ascontiguousarray(
        woT.reshape(KT, P, HID).transpose(1, 0, 2).reshape(P, KT * HID))
    bob = np.broadcast_to(bo.reshape(1, HID), (P, HID)).astype(bf)
    bob = np.ascontiguousarray(bob)

    in_maps = []
    for c in range(N_CORES):
        wqT_c = Wq[c * NH_C * DH:(c + 1) * NH_C * DH].T.astype(bf)
        wqp_c = np.ascontiguousarray(
            wqT_c.reshape(KT, P, NH_C * DH).transpose(1, 0, 2).reshape(
                P, KT * NH_C * DH))
        wkv_c = np.concatenate([Wk[c * DH:(c + 1) * DH],
                                Wv[c * DH:(c + 1) * DH]], axis=0)
        wkvT_c = wkv_c.T.astype(bf)
        wkvp_c = np.ascontiguousarray(
            wkvT_c.reshape(KT, P, 2 * DH).transpose(1, 0, 2).reshape(
                P, KT * 2 * DH))
        bv_c = np.zeros((P, 1), dtype=np.float32)
        bv_c[DH:, 0] = bv[c * DH:(c + 1) * DH]
        in_maps.append({
            "hsp": hsp, "wqp": wqp_c, "wkvp": wkvp_c, "bv": bv_c,
            "cos2": cos2, "sin2": sin2, "rotw": rotw, "identj": identj,
            "tneg": tneg, "ident2": ident2, "vpad": vpad, "wop": wop,
            "bob": bob,
        })

    if _cached_nc is None:
        _cached_nc = _build()
    res = run_bass_kernel_spmd(_cached_nc, in_maps, list(range(N_CORES)))
    last_results = res
    if res.exec_time_ns is not None:
        print(f"HW exec time: {res.exec_time_ns} ns")

    # core e's out rows: 128*pr + 64*q + r -> global 512*(2*pr+q) + 64*e + r
    res_all = np.stack([res.results[c]["out"] for c in range(N_CORES)])
    out = res_all.reshape(N_CORES, 2, 2, DH, HID).transpose(1, 2, 0, 3, 4)
    out = np.ascontiguousarray(out.reshape(1, S, HID))
    return out.astype(np.float32)
